# revision 44
# baseline (speedup 1.0000x reference)
"""Trainium2 Bass kernel for GQA causal attention (nn_Attention).

Reference computation (B=2, S=2048, D=4096, H=32, KV=8, HD=128):
    q/k/v projections -> RoPE(q, k) -> GQA attention with additive mask
    -> softmax -> out projection.

Sharding: TP=4 over heads x DP=2 over batch on 8 NeuronCores.
Each core computes, for its batch b and head shard tp:
    Q^T = (x_b @ wq_tp)^T, K^T, V  (projections with RoPE folded via
    host-side even/odd weight-column reordering + on-device rotation)
    S^T = K^T . Q^T per head (scores, transposed layout)
    P^T = exp(S^T) * expmask_tile   (lazy softmax, no max subtraction)
    U^T = V^T-accumulated P^T; per-query rowsums accumulated on the DVE
    (bf16 pair-folds) and broadcast across partitions with a gpsimd
    partition_all_reduce, so the PE runs no rowsum matmuls at all
    att^T = U^T * (1/rowsum);  out_partial = att @ wo_tp
Host sums the 4 TP partials per batch (the row-parallel all-reduce).

Attention (phase B) and the out-projection (phase C) are emitted
interleaved: B(j=3), B(2), C(3), B(1), C(2), B(0), C(1), C(0).  The C
matmul chains have no scalar-engine dependency, so they keep the PE fed
while the Exp activations (the phase-B pacer) run on the scalar engine.

All matmuls run in bf16 with fp32 PSUM accumulation.  Output is bf16;
the host accumulates the TP partials in fp32.
"""

import os
import math
import numpy as np
import ml_dtypes

# ---------------------------------------------------------------- constants
B, S, D = 2, 2048, 4096
H, KV, HD = 32, 8, 128
N_REP = H // KV
TP, DP = 4, 2
N_CORES = TP * DP
HL = H // TP            # 8 local q heads
KVL = KV // TP          # 2 local kv heads
P = 128                 # partitions
KT = D // P             # 32 contraction tiles for projections
NJ_FULL = S // 512      # 4 seq chunks of 512
NST = S // P            # 16 seq tiles of 128
BF = ml_dtypes.bfloat16

# module-level handle for test harness introspection
last_results = None
_cache = {}


def _classify_mask(mask: np.ndarray):
    """Turn the additive mask into multiplicative per-tile factors.

    Returns (table, uniq) where table[i][j] is 'full' (factor==1
    everywhere), 'zero' (factor==0 everywhere -> tile skipped), or an
    index into uniq, the list of distinct [128,512] f32 factor tiles in
    S^T layout ([sk, sq]).
    """
    m = mask.astype(np.float64)
    rowmax = np.max(m, axis=1, keepdims=True)  # per-query max over keys
    rowmax = np.where(np.isfinite(rowmax), rowmax, 0.0)
    em = np.exp(m - rowmax)                    # [sq, sk] in [0, inf)
    emT = np.ascontiguousarray(em.T).astype(np.float32)  # [sk, sq]
    table = [[None] * NJ_FULL for _ in range(NST)]
    uniq = []
    keys = {}
    for j in range(NJ_FULL):
        first = True
        for i in range(NST):
            t = emT[i * P:(i + 1) * P, j * 512:(j + 1) * 512]
            if np.all(t == 1.0):
                table[i][j] = "full"
                first = False
                continue
            if np.all(t == 0.0):
                table[i][j] = "zero"
                continue
            cols1 = np.all(t == 1.0, axis=0)   # all-ones columns
            cols0 = np.all(t == 0.0, axis=0)   # all-zero columns
            # live range starts after leading all-zero cols (first tile of a
            # j-chunk must start at 0 so the PSUM bank is fully initialized)
            lo = 0
            if not first:
                while lo < 512 and cols0[lo]:
                    lo += 1
            hi = 512
            while hi > lo and cols1[hi - 1]:
                hi -= 1
            w = hi - lo
            sub = t[:, lo:hi]
            key = sub.tobytes()
            if key not in keys:
                keys[key] = len(uniq)
                pad = np.ones((P, 512), np.float32)
                pad[:, :w] = sub
                uniq.append(pad)
            table[i][j] = (lo, w, keys[key])
            first = False
    return table, uniq


def _rope_perm(n_heads):
    """Column permutation putting even rope dims first, odd second, per head."""
    perm = []
    for h in range(n_heads):
        perm += [h * HD + 2 * i for i in range(HD // 2)]
        perm += [h * HD + 2 * i + 1 for i in range(HD // 2)]
    return np.array(perm, dtype=np.int64)


def _build(table_sig, table, n_uniq):
    """Build + compile the SPMD Bass program for one mask classification."""
    import concourse.bass as bass
    import concourse.tile as tile
    import concourse.mybir as mybir
    from concourse import bacc

    bf = mybir.dt.bfloat16
    f32 = mybir.dt.float32
    Exp = mybir.ActivationFunctionType.Exp
    MULT = mybir.AluOpType.mult
    ADD = mybir.AluOpType.add

    nc = bacc.Bacc("TRN2", target_bir_lowering=False, debug=False,
                   enable_asserts=False, num_devices=N_CORES)

    # wq/wk/wv are host-packed to [128, KT*cols] (k-tile-major columns) so
    # each loads with a handful of large DMAs instead of 32 small ones
    VW = KVL * HD
    QW = 2 * HD                    # wq group width (QG heads)
    xT_d = nc.dram_tensor("xT", [D, S], bf, kind="ExternalInput")
    wq_d = nc.dram_tensor("wq", [P, (HL // 2) * KT * QW], bf, kind="ExternalInput")
    wk_d = nc.dram_tensor("wk", [P, KT * VW], bf, kind="ExternalInput")
    wv_d = nc.dram_tensor("wv", [P, KT * VW], bf, kind="ExternalInput")
    wo_d = nc.dram_tensor("wo", [HL * HD, D], bf, kind="ExternalInput")
    cosf_d = nc.dram_tensor("cosf", [P, S], bf, kind="ExternalInput")
    ssf_d = nc.dram_tensor("ssf", [P, S], bf, kind="ExternalInput")
    em_d = [nc.dram_tensor(f"em{u}", [P, 512], bf, kind="ExternalInput")
            for u in range(n_uniq)]
    out_d = nc.dram_tensor("out", [S, D], bf, kind="ExternalOutput")

    with tile.TileContext(nc) as tc:
        with tc.tile_pool(name="consts", bufs=1) as cpool:
            qkv_pool = tc.alloc_tile_pool(name="qkv", bufs=1)
            QT = [qkv_pool.tile([P, S], bf, tag=f"qt{h}", name=f"qt{h}") for h in range(HL)]
            KTt = [qkv_pool.tile([P, S], bf, tag=f"kt{g}", name=f"kt{g}") for g in range(KVL)]
            V = [qkv_pool.tile([P, KVL * HD], bf, tag=f"v{st}", name=f"v{st}") for st in range(NST)]

            # ------------- phase A: projections + RoPE ------------
            XH = 2
            SH = S // XH
            QG = 2                          # q heads per weight group
            NPF = 6               # half-1 x tiles prefetched during half 0
            with tc.tile_pool(name="xt", bufs=1) as xt_pool, \
                 tc.tile_pool(name="xpf", bufs=1) as xpf_pool, \
                 tc.tile_pool(name="wq", bufs=2) as wq_pool, \
                 tc.tile_pool(name="wk", bufs=1) as wk_pool, \
                 tc.tile_pool(name="wv", bufs=1) as wv_pool, \
                 tc.tile_pool(name="ropetmp", bufs=2) as rt_pool, \
                 tc.tile_pool(name="psV", bufs=4, space="PSUM") as psV, \
                 tc.tile_pool(name="psA", bufs=4, space="PSUM") as psA:
                # weights first on the sync queue so the first fused V+K
                # k-loop isn't stuck behind the consts.  One tile per chunk
                # (not one big tile) so each chunk's consumers depend only on
                # its own DMA; small leading chunks arrive fastest.
                CHS = [(0, 4), (4, 8), (8, 16), (16, 24), (24, 32)]
                wv_ch, wk_ch = [], []
                for ci, (k0, k1) in enumerate(CHS):
                    cw = (k1 - k0) * VW
                    tv = wv_pool.tile([P, cw], bf, tag=f"wv{ci}", name=f"wv{ci}")
                    tk = wk_pool.tile([P, cw], bf, tag=f"wk{ci}", name=f"wk{ci}")
                    nc.sync.dma_start(tv[:], wv_d[:, k0 * VW:k1 * VW])
                    nc.sync.dma_start(tk[:], wk_d[:, k0 * VW:k1 * VW])
                    wv_ch.append((k0, tv))
                    wk_ch.append((k0, tk))

                def w_at(chunks, k):
                    for k0, t in reversed(chunks):
                        if k >= k0:
                            return t, (k - k0) * VW
                    raise AssertionError
                # consts on the sync queue after the weights
                cosf = cpool.tile([P, S], bf, tag="cosf", name="cosf")
                ssf = cpool.tile([P, S], bf, tag="ssf", name="ssf")
                nc.sync.dma_start(cosf[:], cosf_d[:, :])
                nc.sync.dma_start(ssf[:], ssf_d[:, :])

                xpf = []              # half-1 prefetch tiles (filled in half 0)
                for half in range(XH):
                    s0 = half * SH
                    xt = []
                    for k in range(KT):
                        if half == 1 and k < NPF:
                            xt.append(xpf[k])
                            continue
                        t = xt_pool.tile([P, SH], bf, tag=f"xt{k}", name=f"xt{k}")
                        # split x tile loads across two DMA queues; scalar
                        # is free at kernel start, sync is free at half 1
                        eng = (nc.gpsimd if (k % 2 == 0)
                               else (nc.scalar if half == 0 else nc.sync))
                        eng.dma_start(t[:], xT_d[k * P:(k + 1) * P, s0:s0 + SH])
                        xt.append(t)

                    def rope_post(dst, ps, jj):
                        qb = rt_pool.tile([P, 512], bf, tag="qb", name="qb")
                        nc.scalar.copy(qb[:], ps[:])
                        qsw = rt_pool.tile([P, 512], bf, tag="qsw", name="qsw")
                        nc.scalar.copy(qsw[0:64, :], ps[64:128, :])
                        nc.scalar.copy(qsw[64:128, :], ps[0:64, :])
                        t1 = rt_pool.tile([P, 512], bf, tag="t1", name="t1")
                        nc.vector.tensor_tensor(
                            t1[:], qb[:], cosf[:, jj * 512:jj * 512 + 512], MULT)
                        t2 = rt_pool.tile([P, 512], bf, tag="t2", name="t2")
                        nc.vector.tensor_tensor(
                            t2[:], qsw[:], ssf[:, jj * 512:jj * 512 + 512], MULT)
                        nc.vector.tensor_tensor(
                            dst[:, jj * 512:jj * 512 + 512], t1[:], t2[:], ADD)

                    def rope_gen(dst, wfn, coff, jj):
                        """dst[:, jj*512..] = rope((x @ w)[:, coff:coff+128])"""
                        ps = psA.tile([P, 512], f32, tag="psqk", name="psqk")
                        lo = jj * 512 - s0
                        for k in range(KT):
                            wt, base = wfn(k)
                            c0 = base + coff
                            nc.tensor.matmul(ps[:], wt[:, c0:c0 + P],
                                             xt[k][:, lo:lo + 512],
                                             start=(k == 0), stop=(k == KT - 1))
                        rope_post(dst, ps, jj)

                    half_js = list(range(half * (NJ_FULL // XH),
                                         (half + 1) * (NJ_FULL // XH)))
                    # V and K first so attention can start early.  The first
                    # V st-group and both kv heads' first K accumulations
                    # share one k-loop so each arriving xt[k] feeds ~850ns of
                    # PE work and the engine stays busy through the x DMA.
                    st0 = half * (NST // XH)
                    # all K chunks of this half fused into the stg0 k-loop:
                    # per-tile PE work (~1.3us) then exceeds DMA supply, so
                    # the loop is never DMA-paced
                    kps = {}
                    for jj in half_js:
                        for g in range(KVL):
                            kps[(g, jj)] = psA.tile([P, 512], f32,
                                                    tag="psqk", name="psqk")
                    for stg in range(2):
                        psv = [psV.tile([P, KVL * HD], f32, tag="psv", name="psv")
                               for _ in range(4)]
                        for k in range(KT):
                            wvt_k, vo = w_at(wv_ch, k)
                            for s4 in range(4):
                                lo = (stg * 4 + s4) * P
                                nc.tensor.matmul(psv[s4][:], xt[k][:, lo:lo + P],
                                                 wvt_k[:, vo:vo + VW],
                                                 start=(k == 0), stop=(k == KT - 1))
                            if stg == 0:
                                wkt_k, ko = w_at(wk_ch, k)
                                for jj in half_js:
                                    klo = jj * 512 - s0
                                    for g in range(KVL):
                                        c0 = ko + g * HD
                                        nc.tensor.matmul(kps[(g, jj)][:],
                                                         wkt_k[:, c0:c0 + P],
                                                         xt[k][:, klo:klo + 512],
                                                         start=(k == 0),
                                                         stop=(k == KT - 1))
                        for s4 in range(4):
                            nc.vector.tensor_copy(V[st0 + stg * 4 + s4][:],
                                                  psv[s4][:])
                    for jj in half_js:
                        for g in range(KVL):
                            rope_post(KTt[g], kps[(g, jj)], jj)
                    if half == 0:
                        # prefetch the first half-1 x tiles during Q work so
                        # the half boundary has no DMA bubble
                        for k in range(NPF):
                            t = xpf_pool.tile([P, SH], bf, tag=f"xp{k}",
                                              name=f"xp{k}")
                            eng = nc.gpsimd if (k % 2 == 0) else nc.scalar
                            eng.dma_start(t[:], xT_d[k * P:(k + 1) * P,
                                                     SH:2 * SH])
                            xpf.append(t)
                    for hg in range(HL // QG):
                        wqt = wq_pool.tile([P, KT * QW], bf, tag="wqg", name="wqg")
                        gw = KT * QW
                        nc.sync.dma_start(wqt[:],
                                          wq_d[:, hg * gw:(hg + 1) * gw])
                        for h in range(hg * QG, (hg + 1) * QG):
                            for jj in half_js:
                                rope_gen(QT[h],
                                         lambda k, t=wqt: (t, k * QW),
                                         (h - hg * QG) * HD, jj)

            # ------------- phases B+C interleaved ------------------
            # mask tiles load first (phase B needs them almost immediately)
            em_pool = tc.alloc_tile_pool(name="em", bufs=1)
            em_sb = []
            for u in range(n_uniq):
                t = em_pool.tile([P, 512], bf, tag=f"em{u}", name=f"em{u}")
                nc.sync.dma_start(t[:], em_d[u][:, :])
                em_sb.append(t)
            wo_pool = tc.alloc_tile_pool(name="wo", bufs=1)
            wot = []
            for h in range(HL):
                t = wo_pool.tile([P, D], bf, tag=f"wo{h}", name=f"wo{h}")
                nc.sync.dma_start(t[:], wo_d[h * P:(h + 1) * P, :])
                wot.append(t)
            ut_pool = tc.alloc_tile_pool(name="ut", bufs=2)
            pt_pool = tc.alloc_tile_pool(name="pt", bufs=3)
            rs_pool = tc.alloc_tile_pool(name="rs", bufs=2)
            ob_pool = tc.alloc_tile_pool(name="ob", bufs=3)
            psX = tc.alloc_tile_pool(name="psX", bufs=3, space="PSUM")
            psU_pool = tc.alloc_tile_pool(name="psU", bufs=2, space="PSUM")
            ones = cpool.tile([P, P], bf, tag="ones", name="ones")
            nc.vector.memset(ones[:], 1.0)

            UTj = {}
            pending_fin = [None]

            def phase_b(j, drip=None):
                """Attention for query chunk j; writes UTj[(j, h)].

                drip: optional generator emitting phase-C matmul quanta;
                pulled between each pair's scores and AV matmuls so the
                PE stays busy while the scalar engine runs Exp.
                """
                inc = [i for i in range(NST) if table[i][j] != "zero"]
                n = len(inc)
                npairs_tot = HL * ((n + 1) // 2)
                rate = (64.0 / npairs_tot) if drip is not None else 0.0
                state = {"carry": 0.0, "drip": drip}

                def pull():
                    state["carry"] += rate
                    while state["carry"] >= 1.0 and state["drip"] is not None:
                        try:
                            next(state["drip"])
                        except StopIteration:
                            state["drip"] = None
                            break
                        state["carry"] -= 1.0

                npairs = (n + 1) // 2
                pairs = []
                for pidx in range(npairs):
                    ia = inc[2 * pidx]
                    ib = inc[2 * pidx + 1] if 2 * pidx + 1 < n else None
                    cls_a = table[ia][j]
                    lo_a = 0 if cls_a == "full" else cls_a[0]
                    cls_b = table[ib][j] if ib is not None else None
                    lo_b = (0 if cls_b == "full" else cls_b[0]) \
                        if ib is not None else 0
                    pairs.append((pidx, ia, ib, cls_a, lo_a, cls_b, lo_b))

                def emit_scores(pd, g, h):
                    pidx, ia, ib, cls_a, lo_a, cls_b, lo_b = pd
                    psSp = psX.tile([P, 1024], f32, tag="pss", name="pss")
                    nc.tensor.matmul(psSp[:, lo_a:512],
                                     KTt[g][:, ia * P:(ia + 1) * P],
                                     QT[h][:, j * 512 + lo_a:j * 512 + 512],
                                     start=True, stop=True)
                    if ib is not None:
                        nc.tensor.matmul(psSp[:, 512 + lo_b:1024],
                                         KTt[g][:, ib * P:(ib + 1) * P],
                                         QT[h][:, j * 512 + lo_b:j * 512 + 512],
                                         start=True, stop=True)
                    return psSp

                def emit_rest(psSp, pd, g, psU, ptsum):
                    pidx, ia, ib, cls_a, lo_a, cls_b, lo_b = pd
                    pt = pt_pool.tile([P, 1024], bf, tag="pt", name="pt")
                    if ib is not None:
                        nc.scalar.activation(pt[:, lo_a:], psSp[:, lo_a:], Exp)
                    else:
                        nc.scalar.activation(pt[:, lo_a:512],
                                             psSp[:, lo_a:512], Exp)
                    # zero the junk regions the rowsum folds would read
                    if lo_a > 0:
                        nc.vector.memset(pt[:, 0:lo_a], 0.0)
                    if ib is not None and lo_b > 0:
                        nc.vector.memset(pt[:, 512:512 + lo_b], 0.0)
                    if cls_a != "full":
                        _, w, u = cls_a
                        nc.vector.tensor_tensor(
                            pt[:, lo_a:lo_a + w], pt[:, lo_a:lo_a + w],
                            em_sb[u][:, 0:w], MULT)
                    if ib is not None and cls_b != "full":
                        _, w, u = cls_b
                        nc.vector.tensor_tensor(
                            pt[:, 512 + lo_b:512 + lo_b + w],
                            pt[:, 512 + lo_b:512 + lo_b + w],
                            em_sb[u][:, 0:w], MULT)
                    # rowsum accumulation on the DVE (bf16 4x ops), emitted
                    # before the drip pull so phase-C copies can't delay it
                    if pidx == 0:
                        if ib is not None:
                            nc.vector.tensor_tensor(ptsum[:], pt[:, 0:512],
                                                    pt[:, 512:1024], ADD)
                        else:
                            nc.vector.tensor_copy(ptsum[:], pt[:, 0:512])
                    else:
                        if ib is not None:
                            fold = rs_pool.tile([P, 512], bf, tag="fold",
                                                name="fold")
                            nc.vector.tensor_tensor(fold[:], pt[:, 0:512],
                                                    pt[:, 512:1024], ADD)
                            nc.vector.tensor_tensor(ptsum[:], ptsum[:],
                                                    fold[:], ADD)
                        else:
                            nc.vector.tensor_tensor(ptsum[:], ptsum[:],
                                                    pt[:, 0:512], ADD)
                    # phase-C matmuls keep the PE busy while Exp runs
                    pull()
                    idx_a = 2 * pidx
                    nc.tensor.matmul(psU[:, lo_a:],
                                     V[ia][:, g * HD:(g + 1) * HD],
                                     pt[:, lo_a:512],
                                     start=(idx_a == 0),
                                     stop=(idx_a == n - 1))
                    if ib is not None:
                        nc.tensor.matmul(psU[:, lo_b:],
                                         V[ib][:, g * HD:(g + 1) * HD],
                                         pt[:, 512 + lo_b:1024],
                                         start=False,
                                         stop=(idx_a + 1 == n - 1))

                for h in range(HL):
                    g = h // N_REP
                    ut = ut_pool.tile([P, 512], bf, tag=f"ut{h}", name=f"ut{h}")
                    UTj[(j, h)] = ut
                    psU = psU_pool.tile([P, 512], f32, tag="psu", name="psu")
                    ptsum = rs_pool.tile([P, 512], bf, tag="ptsum", name="ptsum")
                    # one-pair scores lookahead: psS(p+1) sits between
                    # psS(p) and psU(p) in the PE stream, hiding Exp latency
                    prev = None
                    for pd in pairs:
                        psSp = emit_scores(pd, g, h)
                        if pd[0] == 0 and pending_fin[0] is not None:
                            # finalize the previous head once its successor's
                            # first score pair is in the PE stream
                            pending_fin[0]()
                            pending_fin[0] = None
                        if prev is not None:
                            emit_rest(prev[0], prev[1], g, psU, ptsum)
                        prev = (psSp, pd)
                    emit_rest(prev[0], prev[1], g, psU, ptsum)

                    def fin(psU=psU, ptsum=ptsum, ut=ut):
                        # rowsum broadcast via M=128 ones-matmul (one per
                        # head per chunk); normalizer lands on all partitions
                        psR = psX.tile([P, 1024], f32, tag="pss", name="pss")
                        nc.tensor.matmul(psR[:, 0:512], ones[:, 0:P],
                                         ptsum[:], start=True, stop=True)
                        rec = rs_pool.tile([P, 512], f32, tag="rec", name="rec")
                        nc.vector.reciprocal_approx_fast(rec[:], psR[:, 0:512])
                        nc.vector.tensor_tensor(ut[:], psU[:], rec[:], MULT)
                    pending_fin[0] = fin
                # drain any leftover phase-C work for this block
                while state["drip"] is not None:
                    try:
                        next(state["drip"])
                    except StopIteration:
                        state["drip"] = None

            copy_rr = [0]

            def c_quanta(j):
                """Out projection for query tiles st = 4j..4j+3, yielded in
                4-matmul quanta (64 per block)."""
                for sti in range(4):
                    st = 4 * j + sti
                    for half in range(2):
                        ob = ob_pool.tile([P, 2048], bf, tag="ob", name="ob")
                        for dcp in range(2):
                            psC = psX.tile([P, 1024], f32, tag="pss", name="pss")
                            for sub in range(2):
                                dc = half * 4 + dcp * 2 + sub
                                for hh in (0, 4):
                                    for h in range(hh, hh + 4):
                                        nc.tensor.matmul(
                                            psC[:, sub * 512:(sub + 1) * 512],
                                            UTj[(j, h)][:, sti * P:(sti + 1) * P],
                                            wot[h][:, dc * 512:dc * 512 + 512],
                                            start=(h == 0), stop=(h == HL - 1))
                                    if not (sub == 1 and hh == 4):
                                        yield
                            # alternate copy engine: both have slack here
                            if copy_rr[0] % 2 == 0:
                                nc.vector.tensor_copy(
                                    ob[:, dcp * 1024:(dcp + 1) * 1024], psC[:])
                            else:
                                nc.scalar.copy(
                                    ob[:, dcp * 1024:(dcp + 1) * 1024], psC[:])
                            copy_rr[0] += 1
                            if dcp == 1:
                                eng = nc.gpsimd if (st % 2 == 0) else nc.sync
                                eng.dma_start(
                                    out_d[st * P:(st + 1) * P,
                                          half * 2048:(half + 1) * 2048],
                                    ob[:])
                            yield

            def phase_c(j):
                """Plain out-projection block (no B to interleave with)."""
                if pending_fin[0] is not None:
                    pending_fin[0]()
                    pending_fin[0] = None
                for _ in c_quanta(j):
                    pass

            phase_b(3)
            phase_b(2, drip=c_quanta(3))
            phase_b(1, drip=c_quanta(2))
            phase_b(0, drip=c_quanta(1))
            phase_c(0)

            psU_pool.release()
            psX.release()
            ob_pool.release()
            rs_pool.release()
            pt_pool.release()
            ut_pool.release()
            wo_pool.release()
            em_pool.release()
            qkv_pool.release()

    nc.compile()
    return nc


def kernel(x, freqs_cos, freqs_sin, mask, wq, wk, wv, wo):
    global last_results
    from concourse.bass_utils import run_bass_kernel_spmd

    x = np.asarray(x)
    mask = np.asarray(mask, dtype=np.float32)
    table, uniq = _classify_mask(mask)
    sig = tuple(tuple(r) for r in table), len(uniq)
    key = ("k", sig)
    if key not in _cache:
        _cache[key] = _build(sig, table, len(uniq))
    nc = _cache[key]

    qperm = _rope_perm(H)
    kperm = _rope_perm(KV)
    wq_r = np.asarray(wq)[:, qperm]
    wk_r = (np.asarray(wk) * (1.0 / math.sqrt(HD)))[:, kperm]
    wv_n = np.asarray(wv)
    wo_n = np.asarray(wo)

    cosT = np.asarray(freqs_cos).T.astype(np.float32)     # [64, S]
    sinT = np.asarray(freqs_sin).T.astype(np.float32)
    cosf = np.concatenate([cosT, cosT], axis=0).astype(BF)  # [128, S]
    ssf = np.concatenate([-sinT, sinT], axis=0).astype(BF)

    def pack_w(w2d):
        """[D, C] -> [128, KT*C], k-tile-major columns."""
        Cc = w2d.shape[1]
        return np.ascontiguousarray(
            w2d.reshape(KT, P, Cc).transpose(1, 0, 2).reshape(P, KT * Cc))

    QW = 2 * HD
    in_maps = []
    for c in range(N_CORES):
        b, tp = c // TP, c % TP
        wq_sh = wq_r[:, tp * HL * HD:(tp + 1) * HL * HD]
        wq_pk = np.ascontiguousarray(
            wq_sh.reshape(KT, P, HL // 2, QW).transpose(1, 2, 0, 3)
            .reshape(P, (HL // 2) * KT * QW))
        m = {
            "xT": np.ascontiguousarray(x[b].T).astype(BF),
            "wq": wq_pk.astype(BF),
            "wk": pack_w(wk_r[:, tp * KVL * HD:(tp + 1) * KVL * HD]).astype(BF),
            "wv": pack_w(wv_n[:, tp * KVL * HD:(tp + 1) * KVL * HD]).astype(BF),
            "wo": np.ascontiguousarray(wo_n[tp * HL * HD:(tp + 1) * HL * HD, :]).astype(BF),
            "cosf": cosf, "ssf": ssf,
        }
        for u, t in enumerate(uniq):
            m[f"em{u}"] = t.astype(BF)
        in_maps.append(m)

    trace = bool(os.environ.get("BASS_TRACE"))
    last_results = run_bass_kernel_spmd(
        nc, in_maps, core_ids=list(range(N_CORES)), trace=trace)

    out = np.zeros((B, S, D), dtype=np.float32)
    for c in range(N_CORES):
        out[c // TP] += last_results.results[c]["out"].astype(np.float32)
    return out


# revision 49
# speedup vs baseline: 1.0133x; 1.0133x over previous
"""Trainium2 Bass kernel for GQA causal attention (nn_Attention).

Reference computation (B=2, S=2048, D=4096, H=32, KV=8, HD=128):
    q/k/v projections -> RoPE(q, k) -> GQA attention with additive mask
    -> softmax -> out projection.

Sharding: TP=4 over heads x DP=2 over batch on 8 NeuronCores.
Each core computes, for its batch b and head shard tp:
    Q^T = (x_b @ wq_tp)^T, K^T, V  (projections with RoPE folded via
    host-side even/odd weight-column reordering + on-device rotation)
    S^T = K^T . Q^T per head (scores, transposed layout)
    P^T = exp(S^T) * expmask_tile   (lazy softmax, no max subtraction)
    U^T = V^T-accumulated P^T; per-query rowsums accumulated on the DVE
    (bf16 pair-folds) and broadcast across partitions with a gpsimd
    partition_all_reduce, so the PE runs no rowsum matmuls at all
    att^T = U^T * (1/rowsum);  out_partial = att @ wo_tp
Host sums the 4 TP partials per batch (the row-parallel all-reduce).

Attention (phase B) and the out-projection (phase C) are emitted
interleaved: B(j=3), B(2), C(3), B(1), C(2), B(0), C(1), C(0).  The C
matmul chains have no scalar-engine dependency, so they keep the PE fed
while the Exp activations (the phase-B pacer) run on the scalar engine.

All matmuls run in bf16 with fp32 PSUM accumulation.  Output is bf16;
the host accumulates the TP partials in fp32.
"""

import os
import math
import numpy as np
import ml_dtypes

# ---------------------------------------------------------------- constants
B, S, D = 2, 2048, 4096
H, KV, HD = 32, 8, 128
N_REP = H // KV
TP, DP = 4, 2
N_CORES = TP * DP
HL = H // TP            # 8 local q heads
KVL = KV // TP          # 2 local kv heads
P = 128                 # partitions
KT = D // P             # 32 contraction tiles for projections
NJ_FULL = S // 512      # 4 seq chunks of 512
NST = S // P            # 16 seq tiles of 128
BF = ml_dtypes.bfloat16

# module-level handle for test harness introspection
last_results = None
_cache = {}


def _classify_mask(mask: np.ndarray):
    """Turn the additive mask into multiplicative per-tile factors.

    Returns (table, uniq) where table[i][j] is 'full' (factor==1
    everywhere), 'zero' (factor==0 everywhere -> tile skipped), or an
    index into uniq, the list of distinct [128,512] f32 factor tiles in
    S^T layout ([sk, sq]).
    """
    m = mask.astype(np.float64)
    rowmax = np.max(m, axis=1, keepdims=True)  # per-query max over keys
    rowmax = np.where(np.isfinite(rowmax), rowmax, 0.0)
    em = np.exp(m - rowmax)                    # [sq, sk] in [0, inf)
    emT = np.ascontiguousarray(em.T).astype(np.float32)  # [sk, sq]
    table = [[None] * NJ_FULL for _ in range(NST)]
    uniq = []
    keys = {}
    for j in range(NJ_FULL):
        first = True
        for i in range(NST):
            t = emT[i * P:(i + 1) * P, j * 512:(j + 1) * 512]
            if np.all(t == 1.0):
                table[i][j] = "full"
                first = False
                continue
            if np.all(t == 0.0):
                table[i][j] = "zero"
                continue
            cols1 = np.all(t == 1.0, axis=0)   # all-ones columns
            cols0 = np.all(t == 0.0, axis=0)   # all-zero columns
            # live range starts after leading all-zero cols (first tile of a
            # j-chunk must start at 0 so the PSUM bank is fully initialized)
            lo = 0
            if not first:
                while lo < 512 and cols0[lo]:
                    lo += 1
            hi = 512
            while hi > lo and cols1[hi - 1]:
                hi -= 1
            w = hi - lo
            sub = t[:, lo:hi]
            key = sub.tobytes()
            if key not in keys:
                keys[key] = len(uniq)
                pad = np.ones((P, 512), np.float32)
                pad[:, :w] = sub
                uniq.append(pad)
            table[i][j] = (lo, w, keys[key])
            first = False
    return table, uniq


def _rope_perm(n_heads):
    """Column permutation putting even rope dims first, odd second, per head."""
    perm = []
    for h in range(n_heads):
        perm += [h * HD + 2 * i for i in range(HD // 2)]
        perm += [h * HD + 2 * i + 1 for i in range(HD // 2)]
    return np.array(perm, dtype=np.int64)


def _build(table_sig, table, n_uniq):
    """Build + compile the SPMD Bass program for one mask classification."""
    import concourse.bass as bass
    import concourse.tile as tile
    import concourse.mybir as mybir
    from concourse import bacc

    bf = mybir.dt.bfloat16
    f32 = mybir.dt.float32
    Exp = mybir.ActivationFunctionType.Exp
    MULT = mybir.AluOpType.mult
    ADD = mybir.AluOpType.add

    nc = bacc.Bacc("TRN2", target_bir_lowering=False, debug=False,
                   enable_asserts=False, num_devices=N_CORES)

    # wq/wk/wv are host-packed to [128, KT*cols] (k-tile-major columns) so
    # each loads with a handful of large DMAs instead of 32 small ones
    VW = KVL * HD
    QW = 2 * HD                    # wq group width (QG heads)
    xT_d = nc.dram_tensor("xT", [D, S], bf, kind="ExternalInput")
    wq_d = nc.dram_tensor("wq", [P, (HL // 2) * KT * QW], bf, kind="ExternalInput")
    wk_d = nc.dram_tensor("wk", [P, KT * VW], bf, kind="ExternalInput")
    wv_d = nc.dram_tensor("wv", [P, KT * VW], bf, kind="ExternalInput")
    wo_d = nc.dram_tensor("wo", [HL * HD, D], bf, kind="ExternalInput")
    cosf_d = nc.dram_tensor("cosf", [P, S], bf, kind="ExternalInput")
    ssf_d = nc.dram_tensor("ssf", [P, S], bf, kind="ExternalInput")
    em_d = [nc.dram_tensor(f"em{u}", [P, 512], bf, kind="ExternalInput")
            for u in range(n_uniq)]
    out_d = nc.dram_tensor("out", [S, D], bf, kind="ExternalOutput")

    with tile.TileContext(nc) as tc:
        with tc.tile_pool(name="consts", bufs=1) as cpool:
            qkv_pool = tc.alloc_tile_pool(name="qkv", bufs=1)
            QT = [qkv_pool.tile([P, S], bf, tag=f"qt{h}", name=f"qt{h}") for h in range(HL)]
            KTt = [qkv_pool.tile([P, S], bf, tag=f"kt{g}", name=f"kt{g}") for g in range(KVL)]
            V = [qkv_pool.tile([P, KVL * HD], bf, tag=f"v{st}", name=f"v{st}") for st in range(NST)]

            # ------------- phase A: projections + RoPE ------------
            XH = 2
            SH = S // XH
            QG = 2                          # q heads per weight group
            NPF = 6               # half-1 x tiles prefetched during half 0
            with tc.tile_pool(name="xt", bufs=1) as xt_pool, \
                 tc.tile_pool(name="xpf", bufs=1) as xpf_pool, \
                 tc.tile_pool(name="wq", bufs=2) as wq_pool, \
                 tc.tile_pool(name="wk", bufs=1) as wk_pool, \
                 tc.tile_pool(name="wv", bufs=1) as wv_pool, \
                 tc.tile_pool(name="ropetmp", bufs=2) as rt_pool, \
                 tc.tile_pool(name="psV", bufs=4, space="PSUM") as psV, \
                 tc.tile_pool(name="psA", bufs=4, space="PSUM") as psA:
                # weights first on the sync queue so the first fused V+K
                # k-loop isn't stuck behind the consts.  One tile per chunk
                # (not one big tile) so each chunk's consumers depend only on
                # its own DMA; small leading chunks arrive fastest.
                CHS = [(0, 4), (4, 8), (8, 16), (16, 24), (24, 32)]
                wv_ch, wk_ch = [], []
                for ci, (k0, k1) in enumerate(CHS):
                    cw = (k1 - k0) * VW
                    tv = wv_pool.tile([P, cw], bf, tag=f"wv{ci}", name=f"wv{ci}")
                    tk = wk_pool.tile([P, cw], bf, tag=f"wk{ci}", name=f"wk{ci}")
                    nc.sync.dma_start(tv[:], wv_d[:, k0 * VW:k1 * VW])
                    nc.sync.dma_start(tk[:], wk_d[:, k0 * VW:k1 * VW])
                    wv_ch.append((k0, tv))
                    wk_ch.append((k0, tk))

                def w_at(chunks, k):
                    for k0, t in reversed(chunks):
                        if k >= k0:
                            return t, (k - k0) * VW
                    raise AssertionError
                # consts on the sync queue after the weights
                cosf = cpool.tile([P, S], bf, tag="cosf", name="cosf")
                ssf = cpool.tile([P, S], bf, tag="ssf", name="ssf")
                nc.sync.dma_start(cosf[:], cosf_d[:, :])
                nc.sync.dma_start(ssf[:], ssf_d[:, :])

                xpf = []              # half-1 prefetch tiles (filled in half 0)
                for half in range(XH):
                    s0 = half * SH
                    xt = []
                    for k in range(KT):
                        if half == 1 and k < NPF:
                            xt.append(xpf[k])
                            continue
                        t = xt_pool.tile([P, SH], bf, tag=f"xt{k}", name=f"xt{k}")
                        # split x tile loads across two DMA queues; scalar
                        # is free at kernel start, sync is free at half 1
                        eng = (nc.gpsimd if (k % 2 == 0)
                               else (nc.scalar if half == 0 else nc.sync))
                        eng.dma_start(t[:], xT_d[k * P:(k + 1) * P, s0:s0 + SH])
                        xt.append(t)

                    def rope_post(dst, ps, jj):
                        qb = rt_pool.tile([P, 512], bf, tag="qb", name="qb")
                        nc.scalar.copy(qb[:], ps[:])
                        qsw = rt_pool.tile([P, 512], bf, tag="qsw", name="qsw")
                        nc.scalar.copy(qsw[0:64, :], ps[64:128, :])
                        nc.scalar.copy(qsw[64:128, :], ps[0:64, :])
                        t1 = rt_pool.tile([P, 512], bf, tag="t1", name="t1")
                        nc.vector.tensor_tensor(
                            t1[:], qb[:], cosf[:, jj * 512:jj * 512 + 512], MULT)
                        t2 = rt_pool.tile([P, 512], bf, tag="t2", name="t2")
                        nc.vector.tensor_tensor(
                            t2[:], qsw[:], ssf[:, jj * 512:jj * 512 + 512], MULT)
                        nc.vector.tensor_tensor(
                            dst[:, jj * 512:jj * 512 + 512], t1[:], t2[:], ADD)

                    def rope_gen(dst, wfn, coff, jj):
                        """dst[:, jj*512..] = rope((x @ w)[:, coff:coff+128])"""
                        ps = psA.tile([P, 512], f32, tag="psqk", name="psqk")
                        lo = jj * 512 - s0
                        for k in range(KT):
                            wt, base = wfn(k)
                            c0 = base + coff
                            nc.tensor.matmul(ps[:], wt[:, c0:c0 + P],
                                             xt[k][:, lo:lo + 512],
                                             start=(k == 0), stop=(k == KT - 1))
                        rope_post(dst, ps, jj)

                    half_js = list(range(half * (NJ_FULL // XH),
                                         (half + 1) * (NJ_FULL // XH)))
                    # V and K first so attention can start early.  The first
                    # V st-group and both kv heads' first K accumulations
                    # share one k-loop so each arriving xt[k] feeds ~850ns of
                    # PE work and the engine stays busy through the x DMA.
                    st0 = half * (NST // XH)
                    # all K chunks of this half fused into the stg0 k-loop:
                    # per-tile PE work (~1.3us) then exceeds DMA supply, so
                    # the loop is never DMA-paced
                    kps = {}
                    for jj in half_js:
                        for g in range(KVL):
                            kps[(g, jj)] = psA.tile([P, 512], f32,
                                                    tag="psqk", name="psqk")
                    for stg in range(2):
                        psv = [psV.tile([P, KVL * HD], f32, tag="psv", name="psv")
                               for _ in range(4)]
                        for k in range(KT):
                            wvt_k, vo = w_at(wv_ch, k)
                            for s4 in range(4):
                                lo = (stg * 4 + s4) * P
                                nc.tensor.matmul(psv[s4][:], xt[k][:, lo:lo + P],
                                                 wvt_k[:, vo:vo + VW],
                                                 start=(k == 0), stop=(k == KT - 1))
                            if stg == 0:
                                wkt_k, ko = w_at(wk_ch, k)
                                for jj in half_js:
                                    klo = jj * 512 - s0
                                    for g in range(KVL):
                                        c0 = ko + g * HD
                                        nc.tensor.matmul(kps[(g, jj)][:],
                                                         wkt_k[:, c0:c0 + P],
                                                         xt[k][:, klo:klo + 512],
                                                         start=(k == 0),
                                                         stop=(k == KT - 1))
                        for s4 in range(4):
                            nc.vector.tensor_copy(V[st0 + stg * 4 + s4][:],
                                                  psv[s4][:])
                    for jj in half_js:
                        for g in range(KVL):
                            rope_post(KTt[g], kps[(g, jj)], jj)
                    if half == 0:
                        # prefetch the first half-1 x tiles during Q work so
                        # the half boundary has no DMA bubble
                        for k in range(NPF):
                            t = xpf_pool.tile([P, SH], bf, tag=f"xp{k}",
                                              name=f"xp{k}")
                            eng = nc.gpsimd if (k % 2 == 0) else nc.scalar
                            eng.dma_start(t[:], xT_d[k * P:(k + 1) * P,
                                                     SH:2 * SH])
                            xpf.append(t)
                    for hg in range(HL // QG):
                        wqt = wq_pool.tile([P, KT * QW], bf, tag="wqg", name="wqg")
                        gw = KT * QW
                        nc.sync.dma_start(wqt[:],
                                          wq_d[:, hg * gw:(hg + 1) * gw])
                        for h in range(hg * QG, (hg + 1) * QG):
                            for jj in half_js:
                                rope_gen(QT[h],
                                         lambda k, t=wqt: (t, k * QW),
                                         (h - hg * QG) * HD, jj)

            # ------------- phases B+C interleaved ------------------
            # mask tiles load first (phase B needs them almost immediately)
            em_pool = tc.alloc_tile_pool(name="em", bufs=1)
            em_sb = []
            for u in range(n_uniq):
                t = em_pool.tile([P, 512], bf, tag=f"em{u}", name=f"em{u}")
                nc.sync.dma_start(t[:], em_d[u][:, :])
                em_sb.append(t)
            wo_pool = tc.alloc_tile_pool(name="wo", bufs=1)
            wot = []
            for h in range(HL):
                t = wo_pool.tile([P, D], bf, tag=f"wo{h}", name=f"wo{h}")
                nc.sync.dma_start(t[:], wo_d[h * P:(h + 1) * P, :])
                wot.append(t)
            ut_pool = tc.alloc_tile_pool(name="ut", bufs=2)
            pt_pool = tc.alloc_tile_pool(name="pt", bufs=3)
            rs_pool = tc.alloc_tile_pool(name="rs", bufs=2)
            ob_pool = tc.alloc_tile_pool(name="ob", bufs=3)
            psX = tc.alloc_tile_pool(name="psX", bufs=3, space="PSUM")
            psU_pool = tc.alloc_tile_pool(name="psU", bufs=2, space="PSUM")
            ones = cpool.tile([P, P], bf, tag="ones", name="ones")
            nc.vector.memset(ones[:], 1.0)

            UTj = {}
            pending_fin = []

            def flush_fins(keep=0):
                while len(pending_fin) > keep:
                    pending_fin.pop(0)()

            def phase_b(j, drip=None):
                """Attention for query chunk j; writes UTj[(j, h)].

                drip: optional generator emitting phase-C matmul quanta;
                pulled between each pair's scores and AV matmuls so the
                PE stays busy while the scalar engine runs Exp.
                """
                # the previous block's deferred finalizes must all be emitted
                # before this block's drip reads its UT tiles
                flush_fins()
                inc = [i for i in range(NST) if table[i][j] != "zero"]
                n = len(inc)
                npairs_tot = HL * ((n + 1) // 2)
                rate = (64.0 / npairs_tot) if drip is not None else 0.0
                state = {"carry": 0.0, "drip": drip}

                def pull():
                    state["carry"] += rate
                    while state["carry"] >= 1.0 and state["drip"] is not None:
                        try:
                            next(state["drip"])
                        except StopIteration:
                            state["drip"] = None
                            break
                        state["carry"] -= 1.0

                npairs = (n + 1) // 2
                pairs = []
                for pidx in range(npairs):
                    ia = inc[2 * pidx]
                    ib = inc[2 * pidx + 1] if 2 * pidx + 1 < n else None
                    cls_a = table[ia][j]
                    lo_a = 0 if cls_a == "full" else cls_a[0]
                    cls_b = table[ib][j] if ib is not None else None
                    lo_b = (0 if cls_b == "full" else cls_b[0]) \
                        if ib is not None else 0
                    pairs.append((pidx, ia, ib, cls_a, lo_a, cls_b, lo_b))

                def emit_scores(pd, g, h):
                    pidx, ia, ib, cls_a, lo_a, cls_b, lo_b = pd
                    psSp = psX.tile([P, 1024], f32, tag="pss", name="pss")
                    nc.tensor.matmul(psSp[:, lo_a:512],
                                     KTt[g][:, ia * P:(ia + 1) * P],
                                     QT[h][:, j * 512 + lo_a:j * 512 + 512],
                                     start=True, stop=True)
                    if ib is not None:
                        nc.tensor.matmul(psSp[:, 512 + lo_b:1024],
                                         KTt[g][:, ib * P:(ib + 1) * P],
                                         QT[h][:, j * 512 + lo_b:j * 512 + 512],
                                         start=True, stop=True)
                    return psSp

                def emit_rest(psSp, pd, g, psU, ptsum):
                    pidx, ia, ib, cls_a, lo_a, cls_b, lo_b = pd
                    pt = pt_pool.tile([P, 1024], bf, tag="pt", name="pt")
                    if ib is not None:
                        nc.scalar.activation(pt[:, lo_a:], psSp[:, lo_a:], Exp)
                    else:
                        nc.scalar.activation(pt[:, lo_a:512],
                                             psSp[:, lo_a:512], Exp)
                    # zero the junk regions the rowsum folds would read
                    if lo_a > 0:
                        nc.vector.memset(pt[:, 0:lo_a], 0.0)
                    if ib is not None and lo_b > 0:
                        nc.vector.memset(pt[:, 512:512 + lo_b], 0.0)
                    if cls_a != "full":
                        _, w, u = cls_a
                        nc.vector.tensor_tensor(
                            pt[:, lo_a:lo_a + w], pt[:, lo_a:lo_a + w],
                            em_sb[u][:, 0:w], MULT)
                    if ib is not None and cls_b != "full":
                        _, w, u = cls_b
                        nc.vector.tensor_tensor(
                            pt[:, 512 + lo_b:512 + lo_b + w],
                            pt[:, 512 + lo_b:512 + lo_b + w],
                            em_sb[u][:, 0:w], MULT)
                    # rowsum accumulation on the DVE (bf16 4x ops), emitted
                    # before the drip pull so phase-C copies can't delay it
                    if pidx == 0:
                        if ib is not None:
                            nc.vector.tensor_tensor(ptsum[:], pt[:, 0:512],
                                                    pt[:, 512:1024], ADD)
                        else:
                            nc.vector.tensor_copy(ptsum[:], pt[:, 0:512])
                    else:
                        if ib is not None:
                            fold = rs_pool.tile([P, 512], bf, tag="fold",
                                                name="fold")
                            nc.vector.tensor_tensor(fold[:], pt[:, 0:512],
                                                    pt[:, 512:1024], ADD)
                            nc.vector.tensor_tensor(ptsum[:], ptsum[:],
                                                    fold[:], ADD)
                        else:
                            nc.vector.tensor_tensor(ptsum[:], ptsum[:],
                                                    pt[:, 0:512], ADD)
                    # phase-C matmuls keep the PE busy while Exp runs
                    pull()
                    idx_a = 2 * pidx
                    nc.tensor.matmul(psU[:, lo_a:],
                                     V[ia][:, g * HD:(g + 1) * HD],
                                     pt[:, lo_a:512],
                                     start=(idx_a == 0),
                                     stop=(idx_a == n - 1))
                    if ib is not None:
                        nc.tensor.matmul(psU[:, lo_b:],
                                         V[ib][:, g * HD:(g + 1) * HD],
                                         pt[:, 512 + lo_b:1024],
                                         start=False,
                                         stop=(idx_a + 1 == n - 1))

                for h in range(HL):
                    g = h // N_REP
                    ut = ut_pool.tile([P, 512], bf, tag=f"ut{h}", name=f"ut{h}")
                    UTj[(j, h)] = ut
                    psU = psU_pool.tile([P, 512], f32, tag="psu", name="psu")
                    ptsum = rs_pool.tile([P, 512], bf, tag="ptsum", name="ptsum")
                    # one-pair scores lookahead: psS(p+1) sits between
                    # psS(p) and psU(p) in the PE stream, hiding Exp latency
                    prev = None
                    for pd in pairs:
                        psSp = emit_scores(pd, g, h)
                        if pd[0] == 0:
                            # finalize the head from two iterations back; its
                            # rowsum chain has long finished by now
                            flush_fins(keep=1)
                        if prev is not None:
                            emit_rest(prev[0], prev[1], g, psU, ptsum)
                        prev = (psSp, pd)
                    emit_rest(prev[0], prev[1], g, psU, ptsum)

                    def fin(psU=psU, ptsum=ptsum, ut=ut):
                        # rowsum broadcast via M=128 ones-matmul (one per
                        # head per chunk); normalizer lands on all partitions
                        psR = psX.tile([P, 1024], f32, tag="pss", name="pss")
                        nc.tensor.matmul(psR[:, 0:512], ones[:, 0:P],
                                         ptsum[:], start=True, stop=True)
                        rec = rs_pool.tile([P, 512], f32, tag="rec", name="rec")
                        nc.vector.reciprocal_approx_fast(rec[:], psR[:, 0:512])
                        nc.vector.tensor_tensor(ut[:], psU[:], rec[:], MULT)
                    pending_fin.append(fin)
                # drain any leftover phase-C work for this block
                while state["drip"] is not None:
                    try:
                        next(state["drip"])
                    except StopIteration:
                        state["drip"] = None

            copy_rr = [0]

            def c_quanta(j):
                """Out projection for query tiles st = 4j..4j+3, yielded in
                4-matmul quanta (64 per block)."""
                for sti in range(4):
                    st = 4 * j + sti
                    for half in range(2):
                        ob = ob_pool.tile([P, 2048], bf, tag="ob", name="ob")
                        for dcp in range(2):
                            psC = psX.tile([P, 1024], f32, tag="pss", name="pss")
                            for sub in range(2):
                                dc = half * 4 + dcp * 2 + sub
                                for hh in (0, 4):
                                    for h in range(hh, hh + 4):
                                        nc.tensor.matmul(
                                            psC[:, sub * 512:(sub + 1) * 512],
                                            UTj[(j, h)][:, sti * P:(sti + 1) * P],
                                            wot[h][:, dc * 512:dc * 512 + 512],
                                            start=(h == 0), stop=(h == HL - 1))
                                    if not (sub == 1 and hh == 4):
                                        yield
                            # alternate copy engine: both have slack here
                            if copy_rr[0] % 2 == 0:
                                nc.vector.tensor_copy(
                                    ob[:, dcp * 1024:(dcp + 1) * 1024], psC[:])
                            else:
                                nc.scalar.copy(
                                    ob[:, dcp * 1024:(dcp + 1) * 1024], psC[:])
                            copy_rr[0] += 1
                            if dcp == 1:
                                eng = nc.gpsimd if (st % 2 == 0) else nc.sync
                                eng.dma_start(
                                    out_d[st * P:(st + 1) * P,
                                          half * 2048:(half + 1) * 2048],
                                    ob[:])
                            yield

            def phase_c(j):
                """Plain out-projection block (no B to interleave with)."""
                flush_fins()
                for _ in c_quanta(j):
                    pass

            phase_b(3)
            phase_b(2, drip=c_quanta(3))
            phase_b(1, drip=c_quanta(2))
            phase_b(0, drip=c_quanta(1))
            phase_c(0)

            psU_pool.release()
            psX.release()
            ob_pool.release()
            rs_pool.release()
            pt_pool.release()
            ut_pool.release()
            wo_pool.release()
            em_pool.release()
            qkv_pool.release()

    nc.compile()
    return nc


def kernel(x, freqs_cos, freqs_sin, mask, wq, wk, wv, wo):
    global last_results
    from concourse.bass_utils import run_bass_kernel_spmd

    x = np.asarray(x)
    mask = np.asarray(mask, dtype=np.float32)
    table, uniq = _classify_mask(mask)
    sig = tuple(tuple(r) for r in table), len(uniq)
    key = ("k", sig)
    if key not in _cache:
        _cache[key] = _build(sig, table, len(uniq))
    nc = _cache[key]

    qperm = _rope_perm(H)
    kperm = _rope_perm(KV)
    wq_r = np.asarray(wq)[:, qperm]
    wk_r = (np.asarray(wk) * (1.0 / math.sqrt(HD)))[:, kperm]
    wv_n = np.asarray(wv)
    wo_n = np.asarray(wo)

    cosT = np.asarray(freqs_cos).T.astype(np.float32)     # [64, S]
    sinT = np.asarray(freqs_sin).T.astype(np.float32)
    cosf = np.concatenate([cosT, cosT], axis=0).astype(BF)  # [128, S]
    ssf = np.concatenate([-sinT, sinT], axis=0).astype(BF)

    def pack_w(w2d):
        """[D, C] -> [128, KT*C], k-tile-major columns."""
        Cc = w2d.shape[1]
        return np.ascontiguousarray(
            w2d.reshape(KT, P, Cc).transpose(1, 0, 2).reshape(P, KT * Cc))

    QW = 2 * HD
    in_maps = []
    for c in range(N_CORES):
        b, tp = c // TP, c % TP
        wq_sh = wq_r[:, tp * HL * HD:(tp + 1) * HL * HD]
        wq_pk = np.ascontiguousarray(
            wq_sh.reshape(KT, P, HL // 2, QW).transpose(1, 2, 0, 3)
            .reshape(P, (HL // 2) * KT * QW))
        m = {
            "xT": np.ascontiguousarray(x[b].T).astype(BF),
            "wq": wq_pk.astype(BF),
            "wk": pack_w(wk_r[:, tp * KVL * HD:(tp + 1) * KVL * HD]).astype(BF),
            "wv": pack_w(wv_n[:, tp * KVL * HD:(tp + 1) * KVL * HD]).astype(BF),
            "wo": np.ascontiguousarray(wo_n[tp * HL * HD:(tp + 1) * HL * HD, :]).astype(BF),
            "cosf": cosf, "ssf": ssf,
        }
        for u, t in enumerate(uniq):
            m[f"em{u}"] = t.astype(BF)
        in_maps.append(m)

    trace = bool(os.environ.get("BASS_TRACE"))
    last_results = run_bass_kernel_spmd(
        nc, in_maps, core_ids=list(range(N_CORES)), trace=trace)

    out = np.zeros((B, S, D), dtype=np.float32)
    for c in range(N_CORES):
        out[c // TP] += last_results.results[c]["out"].astype(np.float32)
    return out
